# revision 1
# baseline (speedup 1.0000x reference)
"""LIF (leaky integrate-and-fire) forward scan on 8 Trainium2 cores.

Recurrence per element (b, h), t = 0..T-1, carried state mem/syn/spike:
    reset   = mem * spike                  (spike in {0,1} -> exact)
    mem     = alpha * (mem - reset) + (1-alpha) * syn
    syn     = beta * syn + (1-beta) * x_t
    spike   = (mem >= thr)
Sharding: data-parallel over batch (8 batches per core); no cross-core
communication.

Device plan per core (P=128 partitions, G=32 free -> E=4096 elements):
  - x arrives host-prepped as [NCH, P, G, Tc] (group-major time chunks).
  - syn recurrence is linear -> one tensor_tensor_scan per chunk over a
    group-major [P, G*(Tc+1)] tile; column 0 of each group carries the
    previous chunk's syn in via d0=0 (state = 0*state + carry, exact).
  - y = (1-alpha)*syn is repacked to time-major [P, Tc, G] on the scalar
    engine (strided read, contiguous write).
  - mem chain: one custom fused DVE op per time step,
        mem' = select(mem < thr, mem, 0) * alpha + y
    which matches the reference's rounding (mult, then add, fp32).
  - spikes: one is_ge pass per chunk on gpsimd, DMA'd out time-major.
"""

import sys

if "/opt/trn_rl_repo" not in sys.path:
    sys.path.insert(0, "/opt/trn_rl_repo")

import numpy as np

P = 128
G = 32
B, T, H = 64, 1000, 512
N_CORES = 8
B_LOC = B // N_CORES
E = B_LOC * H
assert E == P * G
TC = 125

_LIF_OP = None
_NC_CACHE = {}


def _register_lif_op():
    """Register the fused LIF step as a custom DVE op (idempotent)."""
    global _LIF_OP
    if _LIF_OP is not None:
        return _LIF_OP
    import concourse.dve_ops as dve_ops
    from concourse.dve_spec import C0, C1, Spec, Src0, Src1, Zero, lower, select
    from concourse.dve_table_gen import dve_ver_for
    from concourse.dve_uop import DveOpSpec

    name = "LIF_STEP_ANT"
    for op in dve_ops.OPS:
        if op.name == name:
            _LIF_OP = op
            return op

    spec = Spec(
        body=select(Src0 < C0, Src0, Zero) * C1 + Src1,
        reference=lambda in0, in1, s0, s1, imm2: (
            np.where(in0 < s0, in0, np.float32(0.0)).astype(np.float32)
            * np.float32(s1)
            + in1
        ).astype(np.float32),
    )
    row = dve_ops._CUSTOM_DVE_ROW_BASE + len(dve_ops.OPS)
    shas = {}
    for ver in ("v3", "v4"):
        try:
            shas[ver] = DveOpSpec(
                name=name, uops=lower(spec, ver=ver), opcode=row, rd1_en=True
            ).sha(ver)
        except Exception:
            pass
    assert dve_ver_for("TRN2") in shas
    op = dve_ops.DveOp(name, spec, subdim=False, uops_sha=shas)
    dve_ops.OPS.append(op)
    dve_ops._SUB_OPCODE_FOR_NAME[name] = row
    dve_ops.CUSTOM_DVE_SPECS[name] = spec
    _LIF_OP = op
    return op


DEFAULT_OPTS = dict(scan_engine="vector", spike_engine="gpsimd",
                    chain_engine="vector", repeat=1, no_chain=False)


def _build(alpha, beta, thr, t_total, tc, opts=None):
    """Build + finalize the per-core bass kernel. alpha/beta/thr are exact
    fp32 values (as python floats)."""
    import concourse.tile as tile
    from concourse import bacc, mybir

    o = dict(DEFAULT_OPTS)
    if opts:
        o.update(opts)
    A = mybir.AluOpType
    f32 = mybir.dt.float32
    nch = t_total // tc
    assert nch * tc == t_total
    w = tc + 1
    lif = _register_lif_op()

    ka = float(np.float32(1.0) - np.float32(alpha))
    kb = float(np.float32(1.0) - np.float32(beta))

    nc = bacc.Bacc("TRN2", target_bir_lowering=False, debug=False)
    x_in = nc.declare_dram_parameter("x_in", [nch, P, tc, G], f32, isOutput=False)
    s_out = nc.declare_dram_parameter("s_out", [nch, P, tc, G], f32, isOutput=True)

    scan_eng = getattr(nc, o["scan_engine"])
    spike_eng = getattr(nc, o["spike_engine"])
    chain_engs = {
        "vector": [nc.vector] * 2, "gpsimd": [nc.gpsimd] * 2,
        "split": [nc.vector, nc.gpsimd],
    }[o["chain_engine"]]

    def body(tc_ctx, cp, xkp, synp, ytp, mp, ssp, beta0, syn_carry):
        m3_prev = None
        for ch in range(nch):
            # time-major chunk: xk[:, (t+1)*G + g] = x at local time t, group g
            xk = xkp.tile([P, w * G], f32, name=f"xk{ch}", tag="xk")
            xk3 = xk.rearrange("p (t g) -> p t g", g=G)
            if not o.get("no_dma_in"):
                nc.sync.dma_start(xk3[:, 1:, :], x_in[ch])
            if not o.get("no_scale"):
                nc.scalar.mul(xk[:, G:], xk[:, G:], kb)
            nc.gpsimd.tensor_copy(xk3[:, 0, :], syn_carry[:])

            syn = synp.tile([P, w * G], f32, name=f"syn{ch}", tag="syn")
            syn3 = syn.rearrange("p (t g) -> p t g", g=G)
            if not o.get("no_scan"):
                for g in range(G):
                    scan_eng.tensor_tensor_scan(
                        out=syn3[:, :, g], data0=beta0[:], data1=xk3[:, :, g],
                        initial=0.0, op0=A.mult, op1=A.add,
                    )
            nc.gpsimd.tensor_copy(syn_carry[:], syn3[:, tc, :])

            yt = ytp.tile([P, tc * G], f32, name=f"yt{ch}", tag="yt")
            yt3 = yt.rearrange("p (t g) -> p t g", g=G)
            if not o.get("no_scale"):
                nc.scalar.mul(yt[:], syn[:, G:], ka)

            m = mp.tile([P, w * G], f32, name=f"m{ch}", tag="m")
            m3 = m.rearrange("p (t g) -> p t g", g=G)
            if ch == 0:
                nc.gpsimd.memset(m3[:, 0, :], 0.0)
            else:
                nc.gpsimd.tensor_copy(m3[:, 0, :], m3_prev[:, tc, :])

            steps = tc if ch < nch - 1 else tc - 1
            if o.get("no_chain"):
                steps = 0
            half = G // 2
            for j in range(steps):
                if chain_engs[0] is chain_engs[1]:
                    chain_engs[0]._custom_dve(
                        lif, out=m3[:, j + 1, :], in0=m3[:, j, :],
                        in1=yt3[:, j, :], s0=thr, s1=float(np.float32(alpha)),
                    )
                else:
                    chain_engs[0]._custom_dve(
                        lif, out=m3[:, j + 1, 0:half], in0=m3[:, j, 0:half],
                        in1=yt3[:, j, 0:half], s0=thr,
                        s1=float(np.float32(alpha)),
                    )
                    chain_engs[1]._custom_dve(
                        lif, out=m3[:, j + 1, half:G], in0=m3[:, j, half:G],
                        in1=yt3[:, j, half:G], s0=thr,
                        s1=float(np.float32(alpha)),
                    )

            s = ssp.tile([P, tc * G], f32, name=f"s{ch}", tag="s")
            if not o.get("no_spike"):
                spike_eng.tensor_scalar(
                    out=s[:], in0=m[:, 0 : tc * G], scalar1=thr, scalar2=None,
                    op0=A.is_ge,
                )
            if not o.get("no_dma_out"):
                nc.sync.dma_start(s_out[ch], s.rearrange("p (t g) -> p t g", g=G))
            m3_prev = m3

    with tile.TileContext(nc) as tc_ctx:
        with (
            tc_ctx.tile_pool(name="consts", bufs=1) as cp,
            tc_ctx.tile_pool(name="xkp", bufs=2) as xkp,
            tc_ctx.tile_pool(name="synp", bufs=2) as synp,
            tc_ctx.tile_pool(name="ytp", bufs=2) as ytp,
            tc_ctx.tile_pool(name="mp", bufs=2) as mp,
            tc_ctx.tile_pool(name="ssp", bufs=2) as ssp,
        ):
            beta0 = cp.tile([P, w], f32, name="beta0")
            nc.gpsimd.memset(beta0[:], float(np.float32(beta)))
            nc.gpsimd.memset(beta0[:, 0:1], 0.0)
            syn_carry = cp.tile([P, G], f32, name="syn_carry")
            nc.gpsimd.memset(syn_carry[:], 0.0)

            args = (tc_ctx, cp, xkp, synp, ytp, mp, ssp, beta0, syn_carry)
            if o["repeat"] == 1:
                body(*args)
            else:
                with tc_ctx.For_i(0, o["repeat"], 1):
                    body(*args)

    nc.finalize()
    return nc


def _get_nc(alpha, beta, thr, t_total=T, tc=TC, opts=None):
    key = (alpha, beta, thr, t_total, tc,
           tuple(sorted((opts or {}).items())))
    if key not in _NC_CACHE:
        _NC_CACHE[key] = _build(alpha, beta, thr, t_total, tc, opts)
    return _NC_CACHE[key]


def _prep_core(xc, tc):
    """(B_LOC, T, H) f32 -> (NCH, P, Tc, G) time-major, e = b*H + h = p*G + g."""
    t_total = xc.shape[1]
    nch = t_total // tc
    xe = xc.transpose(0, 2, 1).reshape(E, t_total)
    return np.ascontiguousarray(xe.reshape(P, G, nch, tc).transpose(2, 0, 3, 1))


def _unprep_core(res, tc):
    """(NCH, P, Tc, G) -> (B_LOC, T, H)."""
    nch = res.shape[0]
    t_total = nch * tc
    xe = res.transpose(1, 3, 0, 2).reshape(E, t_total)
    return xe.reshape(B_LOC, H, t_total).transpose(0, 2, 1)


LAST_RESULT = None


def _run(x, alpha, beta, thr, t_total, tc, trace=False, opts=None):
    global LAST_RESULT
    from concourse.bass_utils import run_bass_kernel_spmd

    nc = _get_nc(alpha, beta, thr, t_total, tc, opts)
    in_maps = [
        {"x_in": _prep_core(x[c * B_LOC : (c + 1) * B_LOC], tc)}
        for c in range(N_CORES)
    ]
    res = run_bass_kernel_spmd(nc, in_maps, list(range(N_CORES)), trace=trace)
    LAST_RESULT = res
    out = np.empty((x.shape[0], t_total, H), np.float32)
    for c in range(N_CORES):
        out[c * B_LOC : (c + 1) * B_LOC] = _unprep_core(res.results[c]["s_out"], tc)
    return out


def kernel(x, decay_constants, threshold):
    x = np.ascontiguousarray(np.asarray(x, dtype=np.float32))
    d = np.asarray(decay_constants, dtype=np.float32)
    alpha = float(np.clip(d[0], np.float32(0.5), np.float32(1.0)))
    beta = float(np.clip(d[1], np.float32(0.5), np.float32(1.0)))
    thr = float(np.float32(np.asarray(threshold)))
    assert x.shape == (B, T, H), x.shape
    return _run(x, alpha, beta, thr, T, TC)



# revision 4
# speedup vs baseline: 8.5365x; 8.5365x over previous
"""LIF (leaky integrate-and-fire) forward on 8 Trainium2 cores.

Reference semantics per element (b, h), t = 0..T-1 (mem/syn/spike carried):
    mem'   = alpha * select(mem < thr, mem, 0) + (1-alpha) * syn
    syn'   = beta * syn + (1-beta) * x_t
    spike' = (mem' >= thr)            (output at every t)

FAST PATH (speculative, provably checked): while no element ever reaches
the threshold, the reset never engages and the dynamics are LINEAR:
    mem_t = sum_{d>=1} Kc[d] * x_{t-d},   Kc = double-exponential kernel.
That is a causal convolution over time -> Toeplitz matmuls on the (idle)
PE engine. Each 125-step output chunk accumulates two 125-contraction
matmuls (previous chunk = lookback pass A, current chunk = pass B), so
every output sees >= 125 steps of exact history; older history is bounded
by tail(125) * max|x|. The device thresholds mem against
(thr - margin) where margin >= truncation + bf16 rounding bound, writing
u8 spikes (DVE is_ge stripe; ACT Relu((mem - thr_eff) * 2^20) stripe --
any nonzero byte means "some element got within margin of threshold").
Host: if ALL bytes are zero, the exact output is provably all-zero ->
return zeros. Otherwise fall back to the exact sequential kernel below.

Sharding: data-parallel over batch (8 batches per core), no cross-core
communication. x ships bf16 [8, 125, 4096] per core; spikes come back u8.

EXACT FALLBACK: the original sequential kernel (tensor_tensor_scan for
syn; fused custom DVE op chain for mem; is_ge for spikes), bit-matching
the reference recurrence.
"""

import sys

if "/opt/trn_rl_repo" not in sys.path:
    sys.path.insert(0, "/opt/trn_rl_repo")

import numpy as np
import ml_dtypes

P = 128
G = 32
B, T, H = 64, 1000, 512
N_CORES = 8
B_LOC = B // N_CORES
E = B_LOC * H
assert E == P * G
TC = 125
NCH = T // TC

_LIF_OP = None
_NC_CACHE = {}

# ---------------------------------------------------------------------------
# Fast path: linear-regime convolution on PE + thresholded u8 spikes
# ---------------------------------------------------------------------------

FAST_OPTS = dict(wd=944, half=2048)
_ROUND_C = 0.006  # bf16(x) + bf16(Kc) + accumulation slack, relative to max|x|
_ACT_SCALE = float(2.0 ** 20)


def _lif_kernel_coeffs(alpha, beta, n):
    """Kc[d], d=0..n-1 (Kc[0]=0) in float64: mem_t = sum_d Kc[d] x_{t-d}."""
    a, b = float(alpha), float(beta)
    kc = np.zeros(n, np.float64)
    if a >= 1.0 or b >= 1.0:
        return kc  # (1-a)(1-b) = 0 -> mem identically 0
    c = (1.0 - a) * (1.0 - b)
    g = 1.0  # g[m] = sum_{j<=m} a^j b^(m-j), m = d-1
    bp = 1.0
    for d in range(1, n):
        kc[d] = c * g
        bp *= b
        g = a * g + bp
    return kc


def _fast_weights(alpha, beta):
    """(SA, SB) bf16 [125,125] stationaries: out[j] += sum_i S[i,j]*x_row[i].

    SB (pass B, current chunk):  SB[i, j] = Kc[j-i]   (0 for j <= i)
    SA (pass A, previous chunk): SA[i, j] = Kc[125 + j - i]
    tail: sum_{d>125} Kc[d] (float64) for the margin bound.
    """
    kc = _lif_kernel_coeffs(alpha, beta, 2 * TC + 1)
    idx = np.arange(TC)
    d_b = idx[None, :] - idx[:, None]  # j - i
    sb = np.where(d_b >= 1, kc[np.clip(d_b, 0, None)], 0.0)
    sa = kc[TC + d_b]
    if alpha >= 1.0 or beta >= 1.0:
        tail = 0.0
    else:
        tail = max(0.0, 1.0 - kc[1 : TC + 1].sum())
    return (
        sa.astype(ml_dtypes.bfloat16),
        sb.astype(ml_dtypes.bfloat16),
        float(tail),
    )


def _build_fast(thr_eff, opts=None):
    """Per-core bass kernel: x (bf16) -> u8 "spike-or-near-threshold" map."""
    import concourse.tile as tile
    from concourse import bacc, mybir

    o = dict(FAST_OPTS)
    if opts:
        o.update(opts)
    A = mybir.AluOpType
    f32 = mybir.dt.float32
    bf16 = mybir.dt.bfloat16
    u8 = mybir.dt.uint8
    half = o["half"]
    nhalf = E // half
    wd = o["wd"]

    nc = bacc.Bacc("TRN2", target_bir_lowering=False, debug=False)
    x_in = nc.declare_dram_parameter("x_in", [NCH, TC, E], bf16, isOutput=False)
    w_in = nc.declare_dram_parameter("w_in", [2, TC, TC], bf16, isOutput=False)
    s_out = nc.declare_dram_parameter("s_out", [NCH, TC, E], u8, isOutput=True)

    with tile.TileContext(nc) as tc_ctx:
        with (
            tc_ctx.tile_pool(name="wp", bufs=1) as wp,
            tc_ctx.tile_pool(name="xp", bufs=3) as xp,
            tc_ctx.tile_pool(name="pp", bufs=2, space="PSUM") as pp,
            tc_ctx.tile_pool(name="sp", bufs=3) as sp,
        ):
            wa = wp.tile([TC, TC], bf16, name="wa")
            wb = wp.tile([TC, TC], bf16, name="wb")
            nc.sync.dma_start(wa[:], w_in[0])
            nc.sync.dma_start(wb[:], w_in[1])
            bias_t = wp.tile([TC, 1], f32, name="bias")
            nc.vector.memset(bias_t[:], -thr_eff * _ACT_SCALE)

            x_prev = None
            for ch in range(NCH):
                xt = xp.tile([TC, E], bf16, name=f"x{ch}", tag="x")
                nc.sync.dma_start(xt[:], x_in[ch])
                for hf in range(nhalf):
                    ps = pp.tile([TC, half], f32, name=f"m{ch}_{hf}", tag="ps")
                    for c in range(half // 512):
                        sl = slice(hf * half + c * 512, hf * half + (c + 1) * 512)
                        out = ps[:, c * 512 : (c + 1) * 512]
                        if ch > 0:
                            nc.tensor.matmul(
                                out=out, lhsT=wa[:], rhs=x_prev[:, sl],
                                start=True, stop=False,
                            )
                        nc.tensor.matmul(
                            out=out, lhsT=wb[:], rhs=xt[:, sl],
                            start=(ch == 0), stop=True,
                        )
                    s = sp.tile([TC, half], u8, name=f"s{ch}_{hf}", tag="s")
                    if wd > 0:
                        nc.vector.tensor_scalar(
                            out=s[:, 0:wd], in0=ps[:, 0:wd],
                            scalar1=thr_eff, scalar2=None, op0=A.is_ge,
                        )
                    if wd < half:
                        nc.scalar.activation(
                            s[:, wd:half], ps[:, wd:half],
                            mybir.ActivationFunctionType.Relu,
                            bias=bias_t[:], scale=_ACT_SCALE,
                        )
                    nc.sync.dma_start(
                        s_out[ch][:, hf * half : (hf + 1) * half], s[:]
                    )
                x_prev = xt

    nc.finalize()
    return nc


def _get_fast_nc(thr_eff, opts=None):
    key = ("fast", thr_eff, tuple(sorted((opts or {}).items())))
    if key not in _NC_CACHE:
        _NC_CACHE[key] = _build_fast(thr_eff, opts)
    return _NC_CACHE[key]


def _prep_fast_core(xc):
    """(B_LOC, T, H) f32 -> (NCH, TC, E) bf16, e = b*H + h, t-major rows."""
    xe = np.ascontiguousarray(xc.transpose(1, 0, 2)).reshape(T, E)
    return xe.reshape(NCH, TC, E).astype(ml_dtypes.bfloat16)


LAST_RESULT = None


def _run_fast(x, alpha, beta, thr_eff, trace=False, opts=None):
    """Returns (any_hot: bool, results list of u8 arrays [NCH, TC, E])."""
    global LAST_RESULT
    from concourse.bass_utils import run_bass_kernel_spmd

    sa, sb, _tail = _fast_weights(alpha, beta)
    w = np.stack([np.asarray(sa), np.asarray(sb)])
    nc = _get_fast_nc(thr_eff, opts)
    in_maps = [
        {"x_in": _prep_fast_core(x[c * B_LOC : (c + 1) * B_LOC]), "w_in": w}
        for c in range(N_CORES)
    ]
    res = run_bass_kernel_spmd(nc, in_maps, list(range(N_CORES)), trace=trace)
    LAST_RESULT = res
    outs = [res.results[c]["s_out"] for c in range(N_CORES)]
    any_hot = any(np.any(o) for o in outs)
    return any_hot, outs


# ---------------------------------------------------------------------------
# Exact fallback: sequential scan kernel (original implementation)
# ---------------------------------------------------------------------------


def _register_lif_op():
    """Register the fused LIF step as a custom DVE op (idempotent)."""
    global _LIF_OP
    if _LIF_OP is not None:
        return _LIF_OP
    import concourse.dve_ops as dve_ops
    from concourse.dve_spec import C0, C1, Spec, Src0, Src1, Zero, lower, select
    from concourse.dve_table_gen import dve_ver_for
    from concourse.dve_uop import DveOpSpec

    name = "LIF_STEP_ANT"
    for op in dve_ops.OPS:
        if op.name == name:
            _LIF_OP = op
            return op

    spec = Spec(
        body=select(Src0 < C0, Src0, Zero) * C1 + Src1,
        reference=lambda in0, in1, s0, s1, imm2: (
            np.where(in0 < s0, in0, np.float32(0.0)).astype(np.float32)
            * np.float32(s1)
            + in1
        ).astype(np.float32),
    )
    row = dve_ops._CUSTOM_DVE_ROW_BASE + len(dve_ops.OPS)
    shas = {}
    for ver in ("v3", "v4"):
        try:
            shas[ver] = DveOpSpec(
                name=name, uops=lower(spec, ver=ver), opcode=row, rd1_en=True
            ).sha(ver)
        except Exception:
            pass
    assert dve_ver_for("TRN2") in shas
    op = dve_ops.DveOp(name, spec, subdim=False, uops_sha=shas)
    dve_ops.OPS.append(op)
    dve_ops._SUB_OPCODE_FOR_NAME[name] = row
    dve_ops.CUSTOM_DVE_SPECS[name] = spec
    _LIF_OP = op
    return op


DEFAULT_OPTS = dict(scan_engine="vector", spike_engine="gpsimd",
                    chain_engine="vector", repeat=1, no_chain=False)


def _build(alpha, beta, thr, t_total, tc, opts=None):
    """Build + finalize the per-core exact bass kernel."""
    import concourse.tile as tile
    from concourse import bacc, mybir

    o = dict(DEFAULT_OPTS)
    if opts:
        o.update(opts)
    A = mybir.AluOpType
    f32 = mybir.dt.float32
    nch = t_total // tc
    assert nch * tc == t_total
    w = tc + 1
    lif = _register_lif_op()

    ka = float(np.float32(1.0) - np.float32(alpha))
    kb = float(np.float32(1.0) - np.float32(beta))

    nc = bacc.Bacc("TRN2", target_bir_lowering=False, debug=False)
    x_in = nc.declare_dram_parameter("x_in", [nch, P, tc, G], f32, isOutput=False)
    s_out = nc.declare_dram_parameter("s_out", [nch, P, tc, G], f32, isOutput=True)

    scan_eng = getattr(nc, o["scan_engine"])
    spike_eng = getattr(nc, o["spike_engine"])
    chain_engs = {
        "vector": [nc.vector] * 2, "gpsimd": [nc.gpsimd] * 2,
        "split": [nc.vector, nc.gpsimd],
    }[o["chain_engine"]]

    def body(tc_ctx, cp, xkp, synp, ytp, mp, ssp, beta0, syn_carry):
        m3_prev = None
        for ch in range(nch):
            xk = xkp.tile([P, w * G], f32, name=f"xk{ch}", tag="xk")
            xk3 = xk.rearrange("p (t g) -> p t g", g=G)
            if not o.get("no_dma_in"):
                nc.sync.dma_start(xk3[:, 1:, :], x_in[ch])
            if not o.get("no_scale"):
                nc.scalar.mul(xk[:, G:], xk[:, G:], kb)
            nc.gpsimd.tensor_copy(xk3[:, 0, :], syn_carry[:])

            syn = synp.tile([P, w * G], f32, name=f"syn{ch}", tag="syn")
            syn3 = syn.rearrange("p (t g) -> p t g", g=G)
            if not o.get("no_scan"):
                for g in range(G):
                    scan_eng.tensor_tensor_scan(
                        out=syn3[:, :, g], data0=beta0[:], data1=xk3[:, :, g],
                        initial=0.0, op0=A.mult, op1=A.add,
                    )
            nc.gpsimd.tensor_copy(syn_carry[:], syn3[:, tc, :])

            yt = ytp.tile([P, tc * G], f32, name=f"yt{ch}", tag="yt")
            yt3 = yt.rearrange("p (t g) -> p t g", g=G)
            if not o.get("no_scale"):
                nc.scalar.mul(yt[:], syn[:, G:], ka)

            m = mp.tile([P, w * G], f32, name=f"m{ch}", tag="m")
            m3 = m.rearrange("p (t g) -> p t g", g=G)
            if ch == 0:
                nc.gpsimd.memset(m3[:, 0, :], 0.0)
            else:
                nc.gpsimd.tensor_copy(m3[:, 0, :], m3_prev[:, tc, :])

            steps = tc if ch < nch - 1 else tc - 1
            if o.get("no_chain"):
                steps = 0
            half = G // 2
            for j in range(steps):
                if chain_engs[0] is chain_engs[1]:
                    chain_engs[0]._custom_dve(
                        lif, out=m3[:, j + 1, :], in0=m3[:, j, :],
                        in1=yt3[:, j, :], s0=thr, s1=float(np.float32(alpha)),
                    )
                else:
                    chain_engs[0]._custom_dve(
                        lif, out=m3[:, j + 1, 0:half], in0=m3[:, j, 0:half],
                        in1=yt3[:, j, 0:half], s0=thr,
                        s1=float(np.float32(alpha)),
                    )
                    chain_engs[1]._custom_dve(
                        lif, out=m3[:, j + 1, half:G], in0=m3[:, j, half:G],
                        in1=yt3[:, j, half:G], s0=thr,
                        s1=float(np.float32(alpha)),
                    )

            s = ssp.tile([P, tc * G], f32, name=f"s{ch}", tag="s")
            if not o.get("no_spike"):
                spike_eng.tensor_scalar(
                    out=s[:], in0=m[:, 0 : tc * G], scalar1=thr, scalar2=None,
                    op0=A.is_ge,
                )
            if not o.get("no_dma_out"):
                nc.sync.dma_start(s_out[ch], s.rearrange("p (t g) -> p t g", g=G))
            m3_prev = m3

    with tile.TileContext(nc) as tc_ctx:
        with (
            tc_ctx.tile_pool(name="consts", bufs=1) as cp,
            tc_ctx.tile_pool(name="xkp", bufs=2) as xkp,
            tc_ctx.tile_pool(name="synp", bufs=2) as synp,
            tc_ctx.tile_pool(name="ytp", bufs=2) as ytp,
            tc_ctx.tile_pool(name="mp", bufs=2) as mp,
            tc_ctx.tile_pool(name="ssp", bufs=2) as ssp,
        ):
            beta0 = cp.tile([P, w], f32, name="beta0")
            nc.gpsimd.memset(beta0[:], float(np.float32(beta)))
            nc.gpsimd.memset(beta0[:, 0:1], 0.0)
            syn_carry = cp.tile([P, G], f32, name="syn_carry")
            nc.gpsimd.memset(syn_carry[:], 0.0)

            args = (tc_ctx, cp, xkp, synp, ytp, mp, ssp, beta0, syn_carry)
            if o["repeat"] == 1:
                body(*args)
            else:
                with tc_ctx.For_i(0, o["repeat"], 1):
                    body(*args)

    nc.finalize()
    return nc


def _get_nc(alpha, beta, thr, t_total=T, tc=TC, opts=None):
    key = (alpha, beta, thr, t_total, tc,
           tuple(sorted((opts or {}).items())))
    if key not in _NC_CACHE:
        _NC_CACHE[key] = _build(alpha, beta, thr, t_total, tc, opts)
    return _NC_CACHE[key]


def _prep_core(xc, tc):
    """(B_LOC, T, H) f32 -> (NCH, P, Tc, G) time-major, e = b*H + h = p*G + g."""
    t_total = xc.shape[1]
    nch = t_total // tc
    xe = xc.transpose(0, 2, 1).reshape(E, t_total)
    return np.ascontiguousarray(xe.reshape(P, G, nch, tc).transpose(2, 0, 3, 1))


def _unprep_core(res, tc):
    """(NCH, P, Tc, G) -> (B_LOC, T, H)."""
    nch = res.shape[0]
    t_total = nch * tc
    xe = res.transpose(1, 3, 0, 2).reshape(E, t_total)
    return xe.reshape(B_LOC, H, t_total).transpose(0, 2, 1)


def _run(x, alpha, beta, thr, t_total=T, tc=TC, trace=False, opts=None):
    """Exact sequential kernel over all cores -> (B, T, H) f32 spikes."""
    global LAST_RESULT
    from concourse.bass_utils import run_bass_kernel_spmd

    nc = _get_nc(alpha, beta, thr, t_total, tc, opts)
    in_maps = [
        {"x_in": _prep_core(x[c * B_LOC : (c + 1) * B_LOC], tc)}
        for c in range(N_CORES)
    ]
    res = run_bass_kernel_spmd(nc, in_maps, list(range(N_CORES)), trace=trace)
    LAST_RESULT = res
    out = np.empty((x.shape[0], t_total, H), np.float32)
    for c in range(N_CORES):
        out[c * B_LOC : (c + 1) * B_LOC] = _unprep_core(res.results[c]["s_out"], tc)
    return out


# ---------------------------------------------------------------------------
# Entry point
# ---------------------------------------------------------------------------


def _margin(x_absmax, alpha, beta):
    _sa, _sb, tail = _fast_weights(alpha, beta)
    return (tail + _ROUND_C) * float(x_absmax) + 1e-6


def kernel(x, decay_constants, threshold, trace=False, opts=None):
    x = np.ascontiguousarray(np.asarray(x, dtype=np.float32))
    d = np.asarray(decay_constants, dtype=np.float32)
    alpha = float(np.clip(d[0], np.float32(0.5), np.float32(1.0)))
    beta = float(np.clip(d[1], np.float32(0.5), np.float32(1.0)))
    thr = float(np.float32(np.asarray(threshold)))
    assert x.shape == (B, T, H), x.shape

    thr_eff = thr - _margin(np.abs(x).max(), alpha, beta)
    if thr_eff > 0.0:
        any_hot, _outs = _run_fast(x, alpha, beta, thr_eff, trace=trace,
                                   opts=opts)
        if not any_hot:
            # Proof: no mem got within `margin` of thr, where margin bounds
            # |mem_fast - mem_exact|; the exact run therefore never spikes.
            return np.zeros((B, T, H), np.float32)
    return _run(x, alpha, beta, thr, T, TC)


# revision 8
# speedup vs baseline: 13.5976x; 1.5929x over previous
"""LIF (leaky integrate-and-fire) forward on 8 Trainium2 cores.

Reference semantics per element (b, h), t = 0..T-1 (mem/syn/spike carried):
    mem'   = alpha * select(mem < thr, mem, 0) + (1-alpha) * syn
    syn'   = beta * syn + (1-beta) * x_t
    spike' = (mem' >= thr)            (output at every t)

FAST PATH (speculative, provably checked): while no element ever reaches
the threshold, the reset never engages and the dynamics are LINEAR:
    mem_t = sum_{d>=1} Kc[d] * x_{t-d},   Kc = double-exponential kernel.
That is a causal convolution over time -> Toeplitz matmuls on the (idle)
PE engine. Each 125-step output chunk accumulates two 125-contraction
matmuls (previous chunk = lookback pass A, current chunk = pass B), so
every output sees >= 125 steps of exact history; older history is bounded
by tail(125) * max|x|. The device thresholds mem against
(thr - margin) where margin >= truncation + bf16 rounding bound, writing
u8 spikes (DVE is_ge stripe; ACT Relu((mem - thr_eff) * 2^20) stripe --
any nonzero byte means "some element got within margin of threshold").
Host: if ALL bytes are zero, the exact output is provably all-zero ->
return zeros. Otherwise fall back to the exact sequential kernel below.

Sharding: data-parallel over batch (8 batches per core), no cross-core
communication. x ships bf16 [8, 125, 4096] per core; spikes come back u8.

EXACT FALLBACK: the original sequential kernel (tensor_tensor_scan for
syn; fused custom DVE op chain for mem; is_ge for spikes), bit-matching
the reference recurrence.
"""

import sys

if "/opt/trn_rl_repo" not in sys.path:
    sys.path.insert(0, "/opt/trn_rl_repo")

import numpy as np
import ml_dtypes

P = 128
G = 32
B, T, H = 64, 1000, 512
N_CORES = 8
B_LOC = B // N_CORES
E = B_LOC * H
assert E == P * G
TC = 125
NCH = T // TC

_LIF_OP = None
_NC_CACHE = {}

# ---------------------------------------------------------------------------
# Fast path: linear-regime convolution on PE + thresholded u8 spikes
# ---------------------------------------------------------------------------

FAST_OPTS = dict(wd=944, half=2048)
_ROUND_C = 0.006  # bf16(x) + bf16(Kc) + accumulation slack, relative to max|x|
_ACT_SCALE = float(2.0 ** 20)


def _lif_kernel_coeffs(alpha, beta, n):
    """Kc[d], d=0..n-1 (Kc[0]=0) in float64: mem_t = sum_d Kc[d] x_{t-d}."""
    a, b = float(alpha), float(beta)
    kc = np.zeros(n, np.float64)
    if a >= 1.0 or b >= 1.0:
        return kc  # (1-a)(1-b) = 0 -> mem identically 0
    c = (1.0 - a) * (1.0 - b)
    g = 1.0  # g[m] = sum_{j<=m} a^j b^(m-j), m = d-1
    bp = 1.0
    for d in range(1, n):
        kc[d] = c * g
        bp *= b
        g = a * g + bp
    return kc


def _fast_weights(alpha, beta):
    """(SA, SB) bf16 [125,128] stationaries: out[j] += sum_i S[i,j]*x_row[i].

    SB (pass B, current chunk):  SB[i, j] = Kc[j-i]   (0 for j <= i)
    SA (pass A, previous chunk): SA[i, j] = Kc[125 + j - i]
    Output columns 125..127 are padding (duplicate mems of the next
    chunk's first steps) so every DMA moves 128 partition rows -- the DGE
    splits a transfer's rows over min(16, divisors) queues, and 125 rows
    would land on only 5 queues.
    tail: sum_{d>125} Kc[d] (float64) for the margin bound.
    """
    kc = _lif_kernel_coeffs(alpha, beta, TC + 130)
    di = np.arange(TC)
    dj = np.arange(128)
    d_b = dj[None, :] - di[:, None]  # j - i
    sb = np.where(d_b >= 1, kc[np.clip(d_b, 0, None)], 0.0)
    sa = kc[TC + d_b]
    if alpha >= 1.0 or beta >= 1.0:
        tail = 0.0
    else:
        tail = max(0.0, 1.0 - kc[1 : TC + 1].sum())
    return (
        sa.astype(ml_dtypes.bfloat16),
        sb.astype(ml_dtypes.bfloat16),
        float(tail),
    )


def _build_fast(thr_eff, opts=None):
    """Per-core bass kernel: x (bf16) -> u8 "spike-or-near-threshold" map."""
    import concourse.tile as tile
    from concourse import bacc, mybir

    o = dict(FAST_OPTS)
    if opts:
        o.update(opts)
    A = mybir.AluOpType
    f32 = mybir.dt.float32
    bf16 = mybir.dt.bfloat16
    u8 = mybir.dt.uint8
    half = o["half"]
    nhalf = E // half
    wd = o["wd"]

    nc = bacc.Bacc("TRN2", target_bir_lowering=False, debug=False)
    x_in = nc.declare_dram_parameter("x_in", [NCH, 128, E], bf16, isOutput=False)
    w_in = nc.declare_dram_parameter("w_in", [2, TC, 128], bf16, isOutput=False)
    s_out = nc.declare_dram_parameter("s_out", [NCH, 128, E], u8, isOutput=True)

    with tile.TileContext(nc) as tc_ctx:
        with (
            tc_ctx.tile_pool(name="wp", bufs=1) as wp,
            tc_ctx.tile_pool(name="xp", bufs=3) as xp,
            tc_ctx.tile_pool(name="pp", bufs=2, space="PSUM") as pp,
            tc_ctx.tile_pool(name="sp", bufs=3) as sp,
        ):
            wa = wp.tile([TC, 128], bf16, name="wa")
            wb = wp.tile([TC, 128], bf16, name="wb")
            nc.sync.dma_start(wa[:], w_in[0])
            nc.sync.dma_start(wb[:], w_in[1])
            bias_t = wp.tile([128, 1], f32, name="bias")
            nc.vector.memset(bias_t[:], -thr_eff * _ACT_SCALE)

            x_prev = None
            for ch in range(NCH):
                xt = xp.tile([128, E], bf16, name=f"x{ch}", tag="x")
                nc.sync.dma_start(xt[:], x_in[ch])
                for hf in range(nhalf):
                    ps = pp.tile([128, half], f32, name=f"m{ch}_{hf}", tag="ps")
                    for c in range(half // 512):
                        sl = slice(hf * half + c * 512, hf * half + (c + 1) * 512)
                        out = ps[:, c * 512 : (c + 1) * 512]
                        if ch > 0:
                            nc.tensor.matmul(
                                out=out, lhsT=wa[:], rhs=x_prev[0:TC, sl],
                                start=True, stop=False,
                            )
                        nc.tensor.matmul(
                            out=out, lhsT=wb[:], rhs=xt[0:TC, sl],
                            start=(ch == 0), stop=True,
                        )
                    s = sp.tile([128, half], u8, name=f"s{ch}_{hf}", tag="s")
                    if wd > 0:
                        nc.vector.tensor_scalar(
                            out=s[:, 0:wd], in0=ps[:, 0:wd],
                            scalar1=thr_eff, scalar2=None, op0=A.is_ge,
                        )
                    if wd < half:
                        nc.scalar.activation(
                            s[:, wd:half], ps[:, wd:half],
                            mybir.ActivationFunctionType.Relu,
                            bias=bias_t[:], scale=_ACT_SCALE,
                        )
                    nc.gpsimd.dma_start(
                        s_out[ch][:, hf * half : (hf + 1) * half], s[:]
                    )
                x_prev = xt

    nc.finalize()
    return nc


def _get_fast_nc(thr_eff, opts=None):
    key = ("fast", thr_eff, tuple(sorted((opts or {}).items())))
    if key not in _NC_CACHE:
        _NC_CACHE[key] = _build_fast(thr_eff, opts)
    return _NC_CACHE[key]


def _prep_fast_core(xc):
    """(B_LOC, T, H) f32 -> (NCH, 128, E) bf16 (rows 125..127 zero pad)."""
    xe = np.ascontiguousarray(xc.transpose(1, 0, 2)).reshape(NCH, TC, E)
    xp = np.zeros((NCH, 128, E), ml_dtypes.bfloat16)
    xp[:, :TC] = xe.astype(ml_dtypes.bfloat16)
    return xp


LAST_RESULT = None


def _run_fast(x, alpha, beta, thr_eff, trace=False, opts=None):
    """Returns (any_hot: bool, results list of u8 arrays [NCH, TC, E])."""
    global LAST_RESULT
    from concourse.bass_utils import run_bass_kernel_spmd

    sa, sb, _tail = _fast_weights(alpha, beta)
    w = np.stack([np.asarray(sa), np.asarray(sb)])
    nc = _get_fast_nc(thr_eff, opts)
    in_maps = [
        {"x_in": _prep_fast_core(x[c * B_LOC : (c + 1) * B_LOC]), "w_in": w}
        for c in range(N_CORES)
    ]
    res = run_bass_kernel_spmd(nc, in_maps, list(range(N_CORES)), trace=trace)
    LAST_RESULT = res
    outs = [res.results[c]["s_out"][:, :TC, :] for c in range(N_CORES)]
    any_hot = any(np.any(o) for o in outs)
    return any_hot, outs


# ---------------------------------------------------------------------------
# Exact fallback: sequential scan kernel (original implementation)
# ---------------------------------------------------------------------------


def _register_lif_op():
    """Register the fused LIF step as a custom DVE op (idempotent)."""
    global _LIF_OP
    if _LIF_OP is not None:
        return _LIF_OP
    import concourse.dve_ops as dve_ops
    from concourse.dve_spec import C0, C1, Spec, Src0, Src1, Zero, lower, select
    from concourse.dve_table_gen import dve_ver_for
    from concourse.dve_uop import DveOpSpec

    name = "LIF_STEP_ANT"
    for op in dve_ops.OPS:
        if op.name == name:
            _LIF_OP = op
            return op

    spec = Spec(
        body=select(Src0 < C0, Src0, Zero) * C1 + Src1,
        reference=lambda in0, in1, s0, s1, imm2: (
            np.where(in0 < s0, in0, np.float32(0.0)).astype(np.float32)
            * np.float32(s1)
            + in1
        ).astype(np.float32),
    )
    row = dve_ops._CUSTOM_DVE_ROW_BASE + len(dve_ops.OPS)
    shas = {}
    for ver in ("v3", "v4"):
        try:
            shas[ver] = DveOpSpec(
                name=name, uops=lower(spec, ver=ver), opcode=row, rd1_en=True
            ).sha(ver)
        except Exception:
            pass
    assert dve_ver_for("TRN2") in shas
    op = dve_ops.DveOp(name, spec, subdim=False, uops_sha=shas)
    dve_ops.OPS.append(op)
    dve_ops._SUB_OPCODE_FOR_NAME[name] = row
    dve_ops.CUSTOM_DVE_SPECS[name] = spec
    _LIF_OP = op
    return op


DEFAULT_OPTS = dict(scan_engine="vector", spike_engine="gpsimd",
                    chain_engine="vector", repeat=1, no_chain=False)


def _build(alpha, beta, thr, t_total, tc, opts=None):
    """Build + finalize the per-core exact bass kernel."""
    import concourse.tile as tile
    from concourse import bacc, mybir

    o = dict(DEFAULT_OPTS)
    if opts:
        o.update(opts)
    A = mybir.AluOpType
    f32 = mybir.dt.float32
    nch = t_total // tc
    assert nch * tc == t_total
    w = tc + 1
    lif = _register_lif_op()

    ka = float(np.float32(1.0) - np.float32(alpha))
    kb = float(np.float32(1.0) - np.float32(beta))

    nc = bacc.Bacc("TRN2", target_bir_lowering=False, debug=False)
    x_in = nc.declare_dram_parameter("x_in", [nch, P, tc, G], f32, isOutput=False)
    s_out = nc.declare_dram_parameter("s_out", [nch, P, tc, G], f32, isOutput=True)

    scan_eng = getattr(nc, o["scan_engine"])
    spike_eng = getattr(nc, o["spike_engine"])
    chain_engs = {
        "vector": [nc.vector] * 2, "gpsimd": [nc.gpsimd] * 2,
        "split": [nc.vector, nc.gpsimd],
    }[o["chain_engine"]]

    def body(tc_ctx, cp, xkp, synp, ytp, mp, ssp, beta0, syn_carry):
        m3_prev = None
        for ch in range(nch):
            xk = xkp.tile([P, w * G], f32, name=f"xk{ch}", tag="xk")
            xk3 = xk.rearrange("p (t g) -> p t g", g=G)
            if not o.get("no_dma_in"):
                nc.sync.dma_start(xk3[:, 1:, :], x_in[ch])
            if not o.get("no_scale"):
                nc.scalar.mul(xk[:, G:], xk[:, G:], kb)
            nc.gpsimd.tensor_copy(xk3[:, 0, :], syn_carry[:])

            syn = synp.tile([P, w * G], f32, name=f"syn{ch}", tag="syn")
            syn3 = syn.rearrange("p (t g) -> p t g", g=G)
            if not o.get("no_scan"):
                for g in range(G):
                    scan_eng.tensor_tensor_scan(
                        out=syn3[:, :, g], data0=beta0[:], data1=xk3[:, :, g],
                        initial=0.0, op0=A.mult, op1=A.add,
                    )
            nc.gpsimd.tensor_copy(syn_carry[:], syn3[:, tc, :])

            yt = ytp.tile([P, tc * G], f32, name=f"yt{ch}", tag="yt")
            yt3 = yt.rearrange("p (t g) -> p t g", g=G)
            if not o.get("no_scale"):
                nc.scalar.mul(yt[:], syn[:, G:], ka)

            m = mp.tile([P, w * G], f32, name=f"m{ch}", tag="m")
            m3 = m.rearrange("p (t g) -> p t g", g=G)
            if ch == 0:
                nc.gpsimd.memset(m3[:, 0, :], 0.0)
            else:
                nc.gpsimd.tensor_copy(m3[:, 0, :], m3_prev[:, tc, :])

            steps = tc if ch < nch - 1 else tc - 1
            if o.get("no_chain"):
                steps = 0
            half = G // 2
            for j in range(steps):
                if chain_engs[0] is chain_engs[1]:
                    chain_engs[0]._custom_dve(
                        lif, out=m3[:, j + 1, :], in0=m3[:, j, :],
                        in1=yt3[:, j, :], s0=thr, s1=float(np.float32(alpha)),
                    )
                else:
                    chain_engs[0]._custom_dve(
                        lif, out=m3[:, j + 1, 0:half], in0=m3[:, j, 0:half],
                        in1=yt3[:, j, 0:half], s0=thr,
                        s1=float(np.float32(alpha)),
                    )
                    chain_engs[1]._custom_dve(
                        lif, out=m3[:, j + 1, half:G], in0=m3[:, j, half:G],
                        in1=yt3[:, j, half:G], s0=thr,
                        s1=float(np.float32(alpha)),
                    )

            s = ssp.tile([P, tc * G], f32, name=f"s{ch}", tag="s")
            if not o.get("no_spike"):
                spike_eng.tensor_scalar(
                    out=s[:], in0=m[:, 0 : tc * G], scalar1=thr, scalar2=None,
                    op0=A.is_ge,
                )
            if not o.get("no_dma_out"):
                nc.sync.dma_start(s_out[ch], s.rearrange("p (t g) -> p t g", g=G))
            m3_prev = m3

    with tile.TileContext(nc) as tc_ctx:
        with (
            tc_ctx.tile_pool(name="consts", bufs=1) as cp,
            tc_ctx.tile_pool(name="xkp", bufs=2) as xkp,
            tc_ctx.tile_pool(name="synp", bufs=2) as synp,
            tc_ctx.tile_pool(name="ytp", bufs=2) as ytp,
            tc_ctx.tile_pool(name="mp", bufs=2) as mp,
            tc_ctx.tile_pool(name="ssp", bufs=2) as ssp,
        ):
            beta0 = cp.tile([P, w], f32, name="beta0")
            nc.gpsimd.memset(beta0[:], float(np.float32(beta)))
            nc.gpsimd.memset(beta0[:, 0:1], 0.0)
            syn_carry = cp.tile([P, G], f32, name="syn_carry")
            nc.gpsimd.memset(syn_carry[:], 0.0)

            args = (tc_ctx, cp, xkp, synp, ytp, mp, ssp, beta0, syn_carry)
            if o["repeat"] == 1:
                body(*args)
            else:
                with tc_ctx.For_i(0, o["repeat"], 1):
                    body(*args)

    nc.finalize()
    return nc


def _get_nc(alpha, beta, thr, t_total=T, tc=TC, opts=None):
    key = (alpha, beta, thr, t_total, tc,
           tuple(sorted((opts or {}).items())))
    if key not in _NC_CACHE:
        _NC_CACHE[key] = _build(alpha, beta, thr, t_total, tc, opts)
    return _NC_CACHE[key]


def _prep_core(xc, tc):
    """(B_LOC, T, H) f32 -> (NCH, P, Tc, G) time-major, e = b*H + h = p*G + g."""
    t_total = xc.shape[1]
    nch = t_total // tc
    xe = xc.transpose(0, 2, 1).reshape(E, t_total)
    return np.ascontiguousarray(xe.reshape(P, G, nch, tc).transpose(2, 0, 3, 1))


def _unprep_core(res, tc):
    """(NCH, P, Tc, G) -> (B_LOC, T, H)."""
    nch = res.shape[0]
    t_total = nch * tc
    xe = res.transpose(1, 3, 0, 2).reshape(E, t_total)
    return xe.reshape(B_LOC, H, t_total).transpose(0, 2, 1)


def _run(x, alpha, beta, thr, t_total=T, tc=TC, trace=False, opts=None):
    """Exact sequential kernel over all cores -> (B, T, H) f32 spikes."""
    global LAST_RESULT
    from concourse.bass_utils import run_bass_kernel_spmd

    nc = _get_nc(alpha, beta, thr, t_total, tc, opts)
    in_maps = [
        {"x_in": _prep_core(x[c * B_LOC : (c + 1) * B_LOC], tc)}
        for c in range(N_CORES)
    ]
    res = run_bass_kernel_spmd(nc, in_maps, list(range(N_CORES)), trace=trace)
    LAST_RESULT = res
    out = np.empty((x.shape[0], t_total, H), np.float32)
    for c in range(N_CORES):
        out[c * B_LOC : (c + 1) * B_LOC] = _unprep_core(res.results[c]["s_out"], tc)
    return out


# ---------------------------------------------------------------------------
# Entry point
# ---------------------------------------------------------------------------


def _margin(x_absmax, alpha, beta):
    _sa, _sb, tail = _fast_weights(alpha, beta)
    return (tail + _ROUND_C) * float(x_absmax) + 1e-6


def kernel(x, decay_constants, threshold, trace=False, opts=None):
    x = np.ascontiguousarray(np.asarray(x, dtype=np.float32))
    d = np.asarray(decay_constants, dtype=np.float32)
    alpha = float(np.clip(d[0], np.float32(0.5), np.float32(1.0)))
    beta = float(np.clip(d[1], np.float32(0.5), np.float32(1.0)))
    thr = float(np.float32(np.asarray(threshold)))
    assert x.shape == (B, T, H), x.shape

    thr_eff = thr - _margin(np.abs(x).max(), alpha, beta)
    if thr_eff > 0.0:
        any_hot, _outs = _run_fast(x, alpha, beta, thr_eff, trace=trace,
                                   opts=opts)
        if not any_hot:
            # Proof: no mem got within `margin` of thr, where margin bounds
            # |mem_fast - mem_exact|; the exact run therefore never spikes.
            return np.zeros((B, T, H), np.float32)
    return _run(x, alpha, beta, thr, T, TC)


# revision 9
# speedup vs baseline: 15.2856x; 1.1241x over previous
"""LIF (leaky integrate-and-fire) forward on 8 Trainium2 cores.

Reference semantics per element (b, h), t = 0..T-1 (mem/syn/spike carried):
    mem'   = alpha * select(mem < thr, mem, 0) + (1-alpha) * syn
    syn'   = beta * syn + (1-beta) * x_t
    spike' = (mem' >= thr)            (output at every t)

FAST PATH (speculative, provably checked): while no element ever reaches
the threshold, the reset never engages and the dynamics are LINEAR:
    mem_t = sum_{d>=1} Kc[d] * x_{t-d},   Kc = double-exponential kernel.
That is a causal convolution over time -> Toeplitz matmuls on the (idle)
PE engine. Each 125-step output chunk accumulates two 125-contraction
matmuls (previous chunk = lookback pass A, current chunk = pass B), so
every output sees >= 125 steps of exact history; older history is bounded
by tail(125) * max|x|. The device thresholds mem against
(thr - margin) where margin >= truncation + bf16 rounding bound, writing
u8 spikes (DVE is_ge stripe; ACT Relu((mem - thr_eff) * 2^20) stripe --
any nonzero byte means "some element got within margin of threshold").
Host: if ALL bytes are zero, the exact output is provably all-zero ->
return zeros. Otherwise fall back to the exact sequential kernel below.

Sharding: data-parallel over batch (8 batches per core), no cross-core
communication. x ships bf16 [8, 125, 4096] per core; spikes come back u8.

EXACT FALLBACK: the original sequential kernel (tensor_tensor_scan for
syn; fused custom DVE op chain for mem; is_ge for spikes), bit-matching
the reference recurrence.
"""

import sys

if "/opt/trn_rl_repo" not in sys.path:
    sys.path.insert(0, "/opt/trn_rl_repo")

import numpy as np
import ml_dtypes

P = 128
G = 32
B, T, H = 64, 1000, 512
N_CORES = 8
B_LOC = B // N_CORES
E = B_LOC * H
assert E == P * G
TC = 125
NCH = T // TC

_LIF_OP = None
_NC_CACHE = {}

# ---------------------------------------------------------------------------
# Fast path: linear-regime convolution on PE + thresholded u8 spikes
# ---------------------------------------------------------------------------

FAST_OPTS = dict(wd=944, half=2048)
_ROUND_C = 0.006  # bf16(x) + bf16(Kc) + accumulation slack, relative to max|x|
_ACT_SCALE = float(2.0 ** 20)


def _lif_kernel_coeffs(alpha, beta, n):
    """Kc[d], d=0..n-1 (Kc[0]=0) in float64: mem_t = sum_d Kc[d] x_{t-d}."""
    a, b = float(alpha), float(beta)
    kc = np.zeros(n, np.float64)
    if a >= 1.0 or b >= 1.0:
        return kc  # (1-a)(1-b) = 0 -> mem identically 0
    c = (1.0 - a) * (1.0 - b)
    g = 1.0  # g[m] = sum_{j<=m} a^j b^(m-j), m = d-1
    bp = 1.0
    for d in range(1, n):
        kc[d] = c * g
        bp *= b
        g = a * g + bp
    return kc


def _fast_weights(alpha, beta):
    """(SA, SB) bf16 [125,128] stationaries: out[j] += sum_i S[i,j]*x_row[i].

    SB (pass B, current chunk):  SB[i, j] = Kc[j-i]   (0 for j <= i)
    SA (pass A, previous chunk): SA[i, j] = Kc[125 + j - i]
    Output columns 125..127 are padding (duplicate mems of the next
    chunk's first steps) so every DMA moves 128 partition rows -- the DGE
    splits a transfer's rows over min(16, divisors) queues, and 125 rows
    would land on only 5 queues.
    tail: sum_{d>125} Kc[d] (float64) for the margin bound.
    """
    kc = _lif_kernel_coeffs(alpha, beta, TC + 130)
    di = np.arange(TC)
    dj = np.arange(128)
    d_b = dj[None, :] - di[:, None]  # j - i
    sb = np.where(d_b >= 1, kc[np.clip(d_b, 0, None)], 0.0)
    sa = kc[TC + d_b]
    if alpha >= 1.0 or beta >= 1.0:
        tail = 0.0
    else:
        tail = max(0.0, 1.0 - kc[1 : TC + 1].sum())
    return (
        sa.astype(ml_dtypes.bfloat16),
        sb.astype(ml_dtypes.bfloat16),
        float(tail),
    )


def _build_fast(thr_eff, opts=None):
    """Per-core bass kernel: x (bf16) -> u8 "spike-or-near-threshold" map."""
    import concourse.tile as tile
    from concourse import bacc, mybir

    o = dict(FAST_OPTS)
    if opts:
        o.update(opts)
    A = mybir.AluOpType
    f32 = mybir.dt.float32
    bf16 = mybir.dt.bfloat16
    u8 = mybir.dt.uint8
    half = o["half"]
    nhalf = E // half
    wd = o["wd"]

    nc = bacc.Bacc("TRN2", target_bir_lowering=False, debug=False)
    x_in = nc.declare_dram_parameter("x_in", [NCH, 128, E], bf16, isOutput=False)
    w_in = nc.declare_dram_parameter("w_in", [2, TC, 128], bf16, isOutput=False)
    s_out = nc.declare_dram_parameter("s_out", [NCH, 128, E], u8, isOutput=True)

    with tile.TileContext(nc) as tc_ctx:
        with (
            tc_ctx.tile_pool(name="wp", bufs=1) as wp,
            tc_ctx.tile_pool(name="xp", bufs=NCH) as xp,
            tc_ctx.tile_pool(name="pp", bufs=2, space="PSUM") as pp,
            tc_ctx.tile_pool(name="sp", bufs=4) as sp,
        ):
            # All x tiles up front: the 8 input DMAs enter the queues
            # immediately and stream ahead of the PE.
            xts = [xp.tile([128, E], bf16, name=f"x{ch}", tag="x")
                   for ch in range(NCH)]
            nc.sync.dma_start(xts[0][:], x_in[0])
            wa = wp.tile([TC, 128], bf16, name="wa")
            wb = wp.tile([TC, 128], bf16, name="wb")
            nc.sync.dma_start(wa[:], w_in[0])
            nc.sync.dma_start(wb[:], w_in[1])
            for ch in range(1, NCH):
                nc.sync.dma_start(xts[ch][:], x_in[ch])
            bias_t = wp.tile([128, 1], f32, name="bias")
            nc.vector.memset(bias_t[:], -thr_eff * _ACT_SCALE)

            x_prev = None
            for ch in range(NCH):
                xt = xts[ch]
                for hf in range(nhalf):
                    ps = pp.tile([128, half], f32, name=f"m{ch}_{hf}", tag="ps")
                    for c in range(half // 512):
                        sl = slice(hf * half + c * 512, hf * half + (c + 1) * 512)
                        out = ps[:, c * 512 : (c + 1) * 512]
                        if ch > 0:
                            nc.tensor.matmul(
                                out=out, lhsT=wa[:], rhs=x_prev[0:TC, sl],
                                start=True, stop=False,
                            )
                        nc.tensor.matmul(
                            out=out, lhsT=wb[:], rhs=xt[0:TC, sl],
                            start=(ch == 0), stop=True,
                        )
                    s = sp.tile([128, half], u8, name=f"s{ch}_{hf}", tag="s")
                    if wd > 0:
                        nc.vector.tensor_scalar(
                            out=s[:, 0:wd], in0=ps[:, 0:wd],
                            scalar1=thr_eff, scalar2=None, op0=A.is_ge,
                        )
                    if wd < half:
                        nc.scalar.activation(
                            s[:, wd:half], ps[:, wd:half],
                            mybir.ActivationFunctionType.Relu,
                            bias=bias_t[:], scale=_ACT_SCALE,
                        )
                    nc.gpsimd.dma_start(
                        s_out[ch][:, hf * half : (hf + 1) * half], s[:]
                    )
                x_prev = xt

    nc.finalize()
    return nc


def _get_fast_nc(thr_eff, opts=None):
    key = ("fast", thr_eff, tuple(sorted((opts or {}).items())))
    if key not in _NC_CACHE:
        _NC_CACHE[key] = _build_fast(thr_eff, opts)
    return _NC_CACHE[key]


def _prep_fast_core(xc):
    """(B_LOC, T, H) f32 -> (NCH, 128, E) bf16 (rows 125..127 zero pad)."""
    xe = np.ascontiguousarray(xc.transpose(1, 0, 2)).reshape(NCH, TC, E)
    xp = np.zeros((NCH, 128, E), ml_dtypes.bfloat16)
    xp[:, :TC] = xe.astype(ml_dtypes.bfloat16)
    return xp


LAST_RESULT = None


def _run_fast(x, alpha, beta, thr_eff, trace=False, opts=None):
    """Returns (any_hot: bool, results list of u8 arrays [NCH, TC, E])."""
    global LAST_RESULT
    from concourse.bass_utils import run_bass_kernel_spmd

    sa, sb, _tail = _fast_weights(alpha, beta)
    w = np.stack([np.asarray(sa), np.asarray(sb)])
    nc = _get_fast_nc(thr_eff, opts)
    in_maps = [
        {"x_in": _prep_fast_core(x[c * B_LOC : (c + 1) * B_LOC]), "w_in": w}
        for c in range(N_CORES)
    ]
    res = run_bass_kernel_spmd(nc, in_maps, list(range(N_CORES)), trace=trace)
    LAST_RESULT = res
    outs = [res.results[c]["s_out"][:, :TC, :] for c in range(N_CORES)]
    any_hot = any(np.any(o) for o in outs)
    return any_hot, outs


# ---------------------------------------------------------------------------
# Exact fallback: sequential scan kernel (original implementation)
# ---------------------------------------------------------------------------


def _register_lif_op():
    """Register the fused LIF step as a custom DVE op (idempotent)."""
    global _LIF_OP
    if _LIF_OP is not None:
        return _LIF_OP
    import concourse.dve_ops as dve_ops
    from concourse.dve_spec import C0, C1, Spec, Src0, Src1, Zero, lower, select
    from concourse.dve_table_gen import dve_ver_for
    from concourse.dve_uop import DveOpSpec

    name = "LIF_STEP_ANT"
    for op in dve_ops.OPS:
        if op.name == name:
            _LIF_OP = op
            return op

    spec = Spec(
        body=select(Src0 < C0, Src0, Zero) * C1 + Src1,
        reference=lambda in0, in1, s0, s1, imm2: (
            np.where(in0 < s0, in0, np.float32(0.0)).astype(np.float32)
            * np.float32(s1)
            + in1
        ).astype(np.float32),
    )
    row = dve_ops._CUSTOM_DVE_ROW_BASE + len(dve_ops.OPS)
    shas = {}
    for ver in ("v3", "v4"):
        try:
            shas[ver] = DveOpSpec(
                name=name, uops=lower(spec, ver=ver), opcode=row, rd1_en=True
            ).sha(ver)
        except Exception:
            pass
    assert dve_ver_for("TRN2") in shas
    op = dve_ops.DveOp(name, spec, subdim=False, uops_sha=shas)
    dve_ops.OPS.append(op)
    dve_ops._SUB_OPCODE_FOR_NAME[name] = row
    dve_ops.CUSTOM_DVE_SPECS[name] = spec
    _LIF_OP = op
    return op


DEFAULT_OPTS = dict(scan_engine="vector", spike_engine="gpsimd",
                    chain_engine="vector", repeat=1, no_chain=False)


def _build(alpha, beta, thr, t_total, tc, opts=None):
    """Build + finalize the per-core exact bass kernel."""
    import concourse.tile as tile
    from concourse import bacc, mybir

    o = dict(DEFAULT_OPTS)
    if opts:
        o.update(opts)
    A = mybir.AluOpType
    f32 = mybir.dt.float32
    nch = t_total // tc
    assert nch * tc == t_total
    w = tc + 1
    lif = _register_lif_op()

    ka = float(np.float32(1.0) - np.float32(alpha))
    kb = float(np.float32(1.0) - np.float32(beta))

    nc = bacc.Bacc("TRN2", target_bir_lowering=False, debug=False)
    x_in = nc.declare_dram_parameter("x_in", [nch, P, tc, G], f32, isOutput=False)
    s_out = nc.declare_dram_parameter("s_out", [nch, P, tc, G], f32, isOutput=True)

    scan_eng = getattr(nc, o["scan_engine"])
    spike_eng = getattr(nc, o["spike_engine"])
    chain_engs = {
        "vector": [nc.vector] * 2, "gpsimd": [nc.gpsimd] * 2,
        "split": [nc.vector, nc.gpsimd],
    }[o["chain_engine"]]

    def body(tc_ctx, cp, xkp, synp, ytp, mp, ssp, beta0, syn_carry):
        m3_prev = None
        for ch in range(nch):
            xk = xkp.tile([P, w * G], f32, name=f"xk{ch}", tag="xk")
            xk3 = xk.rearrange("p (t g) -> p t g", g=G)
            if not o.get("no_dma_in"):
                nc.sync.dma_start(xk3[:, 1:, :], x_in[ch])
            if not o.get("no_scale"):
                nc.scalar.mul(xk[:, G:], xk[:, G:], kb)
            nc.gpsimd.tensor_copy(xk3[:, 0, :], syn_carry[:])

            syn = synp.tile([P, w * G], f32, name=f"syn{ch}", tag="syn")
            syn3 = syn.rearrange("p (t g) -> p t g", g=G)
            if not o.get("no_scan"):
                for g in range(G):
                    scan_eng.tensor_tensor_scan(
                        out=syn3[:, :, g], data0=beta0[:], data1=xk3[:, :, g],
                        initial=0.0, op0=A.mult, op1=A.add,
                    )
            nc.gpsimd.tensor_copy(syn_carry[:], syn3[:, tc, :])

            yt = ytp.tile([P, tc * G], f32, name=f"yt{ch}", tag="yt")
            yt3 = yt.rearrange("p (t g) -> p t g", g=G)
            if not o.get("no_scale"):
                nc.scalar.mul(yt[:], syn[:, G:], ka)

            m = mp.tile([P, w * G], f32, name=f"m{ch}", tag="m")
            m3 = m.rearrange("p (t g) -> p t g", g=G)
            if ch == 0:
                nc.gpsimd.memset(m3[:, 0, :], 0.0)
            else:
                nc.gpsimd.tensor_copy(m3[:, 0, :], m3_prev[:, tc, :])

            steps = tc if ch < nch - 1 else tc - 1
            if o.get("no_chain"):
                steps = 0
            half = G // 2
            for j in range(steps):
                if chain_engs[0] is chain_engs[1]:
                    chain_engs[0]._custom_dve(
                        lif, out=m3[:, j + 1, :], in0=m3[:, j, :],
                        in1=yt3[:, j, :], s0=thr, s1=float(np.float32(alpha)),
                    )
                else:
                    chain_engs[0]._custom_dve(
                        lif, out=m3[:, j + 1, 0:half], in0=m3[:, j, 0:half],
                        in1=yt3[:, j, 0:half], s0=thr,
                        s1=float(np.float32(alpha)),
                    )
                    chain_engs[1]._custom_dve(
                        lif, out=m3[:, j + 1, half:G], in0=m3[:, j, half:G],
                        in1=yt3[:, j, half:G], s0=thr,
                        s1=float(np.float32(alpha)),
                    )

            s = ssp.tile([P, tc * G], f32, name=f"s{ch}", tag="s")
            if not o.get("no_spike"):
                spike_eng.tensor_scalar(
                    out=s[:], in0=m[:, 0 : tc * G], scalar1=thr, scalar2=None,
                    op0=A.is_ge,
                )
            if not o.get("no_dma_out"):
                nc.sync.dma_start(s_out[ch], s.rearrange("p (t g) -> p t g", g=G))
            m3_prev = m3

    with tile.TileContext(nc) as tc_ctx:
        with (
            tc_ctx.tile_pool(name="consts", bufs=1) as cp,
            tc_ctx.tile_pool(name="xkp", bufs=2) as xkp,
            tc_ctx.tile_pool(name="synp", bufs=2) as synp,
            tc_ctx.tile_pool(name="ytp", bufs=2) as ytp,
            tc_ctx.tile_pool(name="mp", bufs=2) as mp,
            tc_ctx.tile_pool(name="ssp", bufs=2) as ssp,
        ):
            beta0 = cp.tile([P, w], f32, name="beta0")
            nc.gpsimd.memset(beta0[:], float(np.float32(beta)))
            nc.gpsimd.memset(beta0[:, 0:1], 0.0)
            syn_carry = cp.tile([P, G], f32, name="syn_carry")
            nc.gpsimd.memset(syn_carry[:], 0.0)

            args = (tc_ctx, cp, xkp, synp, ytp, mp, ssp, beta0, syn_carry)
            if o["repeat"] == 1:
                body(*args)
            else:
                with tc_ctx.For_i(0, o["repeat"], 1):
                    body(*args)

    nc.finalize()
    return nc


def _get_nc(alpha, beta, thr, t_total=T, tc=TC, opts=None):
    key = (alpha, beta, thr, t_total, tc,
           tuple(sorted((opts or {}).items())))
    if key not in _NC_CACHE:
        _NC_CACHE[key] = _build(alpha, beta, thr, t_total, tc, opts)
    return _NC_CACHE[key]


def _prep_core(xc, tc):
    """(B_LOC, T, H) f32 -> (NCH, P, Tc, G) time-major, e = b*H + h = p*G + g."""
    t_total = xc.shape[1]
    nch = t_total // tc
    xe = xc.transpose(0, 2, 1).reshape(E, t_total)
    return np.ascontiguousarray(xe.reshape(P, G, nch, tc).transpose(2, 0, 3, 1))


def _unprep_core(res, tc):
    """(NCH, P, Tc, G) -> (B_LOC, T, H)."""
    nch = res.shape[0]
    t_total = nch * tc
    xe = res.transpose(1, 3, 0, 2).reshape(E, t_total)
    return xe.reshape(B_LOC, H, t_total).transpose(0, 2, 1)


def _run(x, alpha, beta, thr, t_total=T, tc=TC, trace=False, opts=None):
    """Exact sequential kernel over all cores -> (B, T, H) f32 spikes."""
    global LAST_RESULT
    from concourse.bass_utils import run_bass_kernel_spmd

    nc = _get_nc(alpha, beta, thr, t_total, tc, opts)
    in_maps = [
        {"x_in": _prep_core(x[c * B_LOC : (c + 1) * B_LOC], tc)}
        for c in range(N_CORES)
    ]
    res = run_bass_kernel_spmd(nc, in_maps, list(range(N_CORES)), trace=trace)
    LAST_RESULT = res
    out = np.empty((x.shape[0], t_total, H), np.float32)
    for c in range(N_CORES):
        out[c * B_LOC : (c + 1) * B_LOC] = _unprep_core(res.results[c]["s_out"], tc)
    return out


# ---------------------------------------------------------------------------
# Entry point
# ---------------------------------------------------------------------------


def _margin(x_absmax, alpha, beta):
    _sa, _sb, tail = _fast_weights(alpha, beta)
    return (tail + _ROUND_C) * float(x_absmax) + 1e-6


def kernel(x, decay_constants, threshold, trace=False, opts=None):
    x = np.ascontiguousarray(np.asarray(x, dtype=np.float32))
    d = np.asarray(decay_constants, dtype=np.float32)
    alpha = float(np.clip(d[0], np.float32(0.5), np.float32(1.0)))
    beta = float(np.clip(d[1], np.float32(0.5), np.float32(1.0)))
    thr = float(np.float32(np.asarray(threshold)))
    assert x.shape == (B, T, H), x.shape

    thr_eff = thr - _margin(np.abs(x).max(), alpha, beta)
    if thr_eff > 0.0:
        any_hot, _outs = _run_fast(x, alpha, beta, thr_eff, trace=trace,
                                   opts=opts)
        if not any_hot:
            # Proof: no mem got within `margin` of thr, where margin bounds
            # |mem_fast - mem_exact|; the exact run therefore never spikes.
            return np.zeros((B, T, H), np.float32)
    return _run(x, alpha, beta, thr, T, TC)
